# revision 65
# baseline (speedup 1.0000x reference)
"""ConvAttention Trainium2 kernel (v7, ~83us vs 101us v5 baseline).

Per-core (data-parallel over batch, 8 cores, 1 image each):
  q/k/v = depthwise 3x3 conv over x [56,56,64], then full attention over
  N=3136 tokens with softmax(q.k * 8), then ctx @ Wp + bp.

Layout:
  - x staged on host as two [128, 58, 58] images (xpT row-shift pair, x3
    col-shift pair) in f32r; convs are tap-stacked K=128 matmuls (5 per
    conv tile); Wp/bv/bp fold into the v-conv; bk dropped (cancels).
  - The diag conv lhsT blocks are built ON-DEVICE (affine_select mask x
    per-partition tap scalars) so the serial-DMA front carries only 48B
    of taps + the dense v-projection blocks before the x row chunks.
  - exp is split across engines: 20 k-chunks/tile on ACT (10 groups of 2
    double-buffered in psSa/psSb), 5 chunks on DVE as a Schraudolph bf16
    exp (i16 = rint(8s*A + B) bitcast to bf16, ~3% elem error, scattered
    chunks so the softmax averages it out; measured 9.1e-3 final err).
    The DVE chunks flow through a dedicated 1-bank psSd score lane +
    separate pTd tiles because the tile scheduler linearizes all
    consumers of a shared PSUM tile with cross-engine semaphores.
  - AV uses pT chunks as lhsT into [token, embed] accumulators with a
    ones column for the softmax denominator (reciprocal + mul norm).
  - Last q-tile's AV interleaves under its own exp groups (subs 0/1
    chunk-wise in the idle conv-psum slots, subs 2/3 catch up in the
    freed psSa/psC slots), and per-tile stores go out as two gathered
    half-tile DMAs, so only ~4us of tail remains after the final exp.
  - PE p-state warmup: pe_busy_start is sticky, so two early matmuls on
    a DVE-memset tile start the clock ramp during the DMA wait.
"""

import sys

import numpy as np

if "/opt/trn_rl_repo" not in sys.path:
    sys.path.insert(0, "/opt/trn_rl_repo")

H = 56
W = 56
C = 64
E = 64
N = H * W               # 3136 tokens
HP = H + 2              # padded
WP = W + 2
NQ = 448                # q-tile (8 spatial rows)
NQT = N // NQ           # 7
KC = 128                # k-chunk (partition dim of s^T tiles)
NKC = (N + KC - 1) // KC  # 25 (last chunk is 64 real tokens)
NPAD = NKC * KC         # 3200 (k padded with zeros)
NCORES = 8

# exp chunk assignment per tile: 20 chunks on ACT in 10 double-buffered
# groups of 2 (psSa/psSb, 2 banks each), 5 chunks on DVE via Schraudolph
# from a dedicated 1-bank score lane (psSd) so the scheduler never chains
# the ACT exp behind the DVE read of a shared tile
DVE_CH = (4, 9, 14, 19, 24)
ACT_GRP = [[0, 1], [2, 3], [5, 6], [7, 8], [10, 11],
           [12, 13], [15, 16], [17, 18], [20, 21], [22, 23]]
NGRP = len(ACT_GRP)
# dve chunk emitted right after this group's QK
DVE_AFTER = {1: 0, 3: 1, 5: 2, 7: 3, 9: 4}
# chunks covered by emission step g (group + trailing dve chunk)
STEP = [ACT_GRP[g] + ([DVE_CH[DVE_AFTER[g]]] if g in DVE_AFTER else [])
        for g in range(NGRP)]
# chunk -> ("act", g, j) or ("dve", i)
CH_LOC = {}
for _g, _js in enumerate(ACT_GRP):
    for _j, _kc in enumerate(_js):
        CH_LOC[_kc] = ("act", _g, _j)
for _i, _kc in enumerate(DVE_CH):
    CH_LOC[_kc] = ("dve", _i)
# kv-conv tile that must be complete before QK of step g (any q-tile)
KV_NEED = [min(((max(STEP[g]) + 1) * KC - 1) // NQ, NQT - 1)
           for g in range(NGRP)]
# stacked conv slots: (lower tap, upper tap or None); taps t = 3*i + j.
# Slots 0-2 pair rows 0+1 via xpT's row-shifted upper half; slot 3 pairs
# (2,0)+(2,1) via x3's col-shifted upper half; slot 4 is the single (2,2).
CONV_SLOTS = [(0, 3), (1, 4), (2, 5), (6, 7), (8, None)]
# emission order: the single K=64 tap first (fewest dependencies)
SLOT_ORDER = (4, 0, 1, 2, 3)

# Schraudolph bf16 exp constants: i16 = trunc((8*s)*A + B) bitcast to
# bf16 approximates exp(8*s) with ~3% element error; the scattered
# DVE_CH pattern averages it out in the softmax (measured ~9.6e-3 final)
SCH_A = 8.0 * 128.0 / 0.6931471805599453
SCH_B = 127.0 * 128.0 - 5.0

_CACHE = {}


def _prep_x(xi):
    """Host staging: [56,56,64] -> (xpT, x3) [128, HP, WP] float32."""
    base = np.zeros((C, HP, WP), np.float32)
    base[:, 1:1 + H, 1:1 + W] = np.ascontiguousarray(xi.transpose(2, 0, 1))
    xp = np.zeros((128, HP, WP), np.float32)
    xp[0:C] = base
    xp[C:128, 0:HP - 1] = base[:, 1:HP]
    x3 = np.zeros((128, HP, WP), np.float32)
    x3[0:C] = base
    x3[C:128, :, 0:WP - 1] = base[:, :, 1:WP]
    return xp, x3


def _prep_weights(wq, wk, wv, bq, bv, Wp, bp):
    """Host staging of the conv weights (diag blocks built on-device).

    wtap [128, 2, 5]: per-partition tap values, [:,0,s]=wq, [:,1,s]=wk
      (partition p<64: lower tap of slot s for channel p; p>=64: upper tap).
    stv [128, 64, 5]: the dense v-projection blocks wv[tap]*Wp, laid out
      [p, out_col, slot] to sit contiguously behind the on-device diag.
    bqb1 [128, 2]: col 0 rows 0-63 = bq; col 1 rows 64-127 = bv@Wp + bp.
    """
    wq = np.asarray(wq, np.float32).reshape(9, C)
    wk = np.asarray(wk, np.float32).reshape(9, C)
    wv = np.asarray(wv, np.float32).reshape(9, C)
    Wp = np.asarray(Wp, np.float32)
    wtap = np.zeros((128, 2, 6), np.float32)
    stv = np.zeros((128, C, 5), np.float32)
    for s, (lt, ut) in enumerate(CONV_SLOTS):
        wtap[0:C, 0, s] = wq[lt]
        wtap[0:C, 1, s] = wk[lt]
        stv[0:C, :, s] = wv[lt][:, None] * Wp
        if ut is not None:
            wtap[C:128, 0, s] = wq[ut]
            wtap[C:128, 1, s] = wk[ut]
            stv[C:128, :, s] = wv[ut][:, None] * Wp
    # biases ride along in wtap[:, :, 5]: [0:C, 0, 5] = bq,
    # [C:128, 1, 5] = bv@Wp + bp
    wtap[0:C, 0, 5] = np.asarray(bq, np.float32)
    wtap[C:128, 1, 5] = (np.asarray(bv, np.float32) @ Wp
                         + np.asarray(bp, np.float32))
    return wtap, stv


def _build(level=99):
    import concourse.bacc as bacc
    import concourse.tile as tile
    from concourse import mybir
    from concourse.masks import make_identity
    from concourse.tile import add_dep_helper

    F32 = mybir.dt.float32
    F32R = mybir.dt.float32r
    BF16 = mybir.dt.bfloat16
    AF = mybir.ActivationFunctionType

    nc = bacc.Bacc(None, target_bir_lowering=False, debug=False)

    x_d = nc.dram_tensor("x", [128, HP, WP], F32R, kind="ExternalInput")
    x3_d = nc.dram_tensor("x3", [128, HP, WP], F32R, kind="ExternalInput")
    wtap_d = nc.dram_tensor("wtap", [128, 2, 6], F32, kind="ExternalInput")
    stv_d = nc.dram_tensor("stv", [128, C, 5], F32R, kind="ExternalInput")
    # per-tile padded output: token qt*448 + s*128 + p -> out_d[qt, s*128+p]
    # (s=3 rows 64-127 are never written; host slices them off)
    out_d = nc.dram_tensor("out", [NQT, 4 * 128, E], F32, kind="ExternalOutput")

    # row-chunked image loads, sized for the serial DMA model: rows 0-9 for
    # tile 0's convs up front, then uniform 8-row chunks just ahead of the
    # kv-conv demand curve. x3 rows 0-1 are never read (conv slot 3 reads
    # rows 8ct+2..8ct+9), so its first chunk starts at row 2.
    RCH = [10, 18, 26, 34, 42, 50, HP]

    with tile.TileContext(nc) as tc:
        with tc.tile_pool(name="const", bufs=1) as const, \
             tc.tile_pool(name="big", bufs=1) as big:
            # warmup tile first: DVE memset -> PE p-state ramp starts ~1us in
            # (memset of f32r is not a valid ISA op; memset as f32, matmul
            # reads it through an f32r bitcast)
            wtile = const.tile([128, 128], F32)
            nc.vector.memset(wtile[:], 0.0)

            xpT = big.tile([128, HP, WP], F32R)
            x3 = big.tile([128, HP, WP], F32R)
            wtap = const.tile([128, 2, 6], F32)
            # [p, out_col, slot]: cols 0-63 diag (on-device), v block via DMA
            st_q = const.tile([128, C, 5], F32R)
            st_kv = const.tile([128, 128, 5], F32R)

            # DMA plan: single (sync) queue; the shared DMA engine device is
            # serial in the cost model and HWDGE descriptor generation costs
            # 625ns per dma_start, so both order and COUNT of front DMAs
            # matter. The diag lhsT blocks are built on-device (wtap is 48B
            # vs 3.8KB staged) and the biases ride in wtap, keeping the
            # serial front at 4 descriptors before the row chunks.
            nc.sync.dma_start(wtap[:], wtap_d[:])
            nc.sync.dma_start(xpT[:, 0:RCH[0], :], x_d[:, 0:RCH[0], :])
            nc.sync.dma_start(st_kv[:, C:128, :], stv_d[:])
            nc.sync.dma_start(x3[:, 2:RCH[0], :], x3_d[:, 2:RCH[0], :])
            for ci in range(len(RCH) - 1):
                r0, r1 = RCH[ci], RCH[ci + 1]
                nc.sync.dma_start(xpT[:, r0:r1, :], x_d[:, r0:r1, :])
                nc.sync.dma_start(x3[:, r0:r1, :], x3_d[:, r0:r1, :])

            ident_f = const.tile([128, 128], F32)
            make_identity(nc, ident_f[:])
            ident_b = const.tile([128, 128], BF16)
            nc.vector.tensor_copy(ident_b[:], ident_f[:])

            # two-half diagonal mask M[p, y] = (y == p mod 64) for the
            # on-device diag builds
            dmask = const.tile([128, C], F32)
            nc.gpsimd.memset(dmask[:], 0.0)
            for half in (0, 1):
                nc.gpsimd.affine_select(
                    out=dmask[half * C:(half + 1) * C, :],
                    in_=dmask[half * C:(half + 1) * C, :],
                    compare_op=mybir.AluOpType.not_equal,
                    fill=1.0,
                    base=0,
                    pattern=[[-1, C]],
                    channel_multiplier=1,
                )
            # st_q / st_kv diag blocks: M * per-partition tap value
            for s in range(5):
                nc.vector.tensor_scalar_mul(
                    st_q[:, :, s], dmask[:], wtap[:, 0, s:s + 1])
                nc.vector.tensor_scalar_mul(
                    st_kv[:, 0:C, s], dmask[:], wtap[:, 1, s:s + 1])

            zsc = const.tile([128, 128], F32)
            nc.vector.memset(zsc[:], 0.0)
            ones_f = const.tile([128, NKC], F32)
            nc.vector.memset(ones_f[:], 1.0)

            qT = big.tile([C, N], F32R)            # q^T  [c, token]
            kT = big.tile([C, NPAD], F32R)         # k^T  [c, token], zero pad
            vT = big.tile([128, N], BF16)          # v''^T on partitions 64-127
            v_nat = big.tile([128, NKC, C + 1], BF16)  # [tok%128, chunk, e|1]

            nc.vector.tensor_copy(kT[:, N:NPAD], zsc[0:C, 0:NPAD - N])
            nc.vector.tensor_copy(v_nat[:, :, C], ones_f[:])

            with tc.tile_pool(name="ps2", bufs=2, space="PSUM") as ps2, \
                 tc.tile_pool(name="psSa", bufs=1, space="PSUM") as psSa, \
                 tc.tile_pool(name="psSb", bufs=1, space="PSUM") as psSb, \
                 tc.tile_pool(name="psSd", bufs=1, space="PSUM") as psSd, \
                 tc.tile_pool(name="psC", bufs=1, space="PSUM") as psC, \
                 tc.tile_pool(name="sbP", bufs=2 * NGRP) as sbP, \
                 tc.tile_pool(name="sbPd", bufs=2 * len(DVE_CH)) as sbPd, \
                 tc.tile_pool(name="sbO", bufs=4) as sbO, \
                 tc.tile_pool(name="sbI", bufs=4) as sbI:

                # PE warmup: set pe_busy_start early so the clock ramps to
                # full rate during the image DMA wait
                # pe_busy_start is sticky from the first matmul (no reset on
                # idle), so two early matmuls are enough to start the clock
                # ramp; the PE hits full rate ~3us later regardless of idling
                ptw = ps2.tile([128, NQ], F32, name="ptw", tag="cv")
                for _ in range(2):
                    nc.tensor.matmul(ptw[0:128, 0:128],
                                     wtile[:, 0:128].bitcast(F32R),
                                     wtile[:, 0:128].bitcast(F32R),
                                     start=True, stop=True)

                # ---- incremental emitters ----------------------------------
                st = {"kv": 0, "vn": 0, "pctx": None}

                def conv_matmuls(pdst, lhsT, ct, mwid, slots=range(5)):
                    r0 = ct * 8
                    first = None
                    for i in slots:
                        s = SLOT_ORDER[i]
                        if s < 3:
                            rhs = xpT[:, r0:r0 + 8, s:s + W]
                            lh = lhsT[:, 0:mwid, s]
                        elif s == 3:
                            rhs = x3[:, r0 + 2:r0 + 10, 0:W]
                            lh = lhsT[:, 0:mwid, s]
                        else:
                            rhs = xpT[0:C, r0 + 2:r0 + 10, 2:2 + W]
                            lh = lhsT[0:C, 0:mwid, s]
                        mm = nc.tensor.matmul(pdst[:], lh, rhs,
                                              start=(i == 0), stop=(i == 4))
                        if first is None:
                            first = mm
                    return first

                def emit_vt_add(pkv, ct):
                    nc.vector.tensor_scalar_add(
                        vT[C:128, ct * NQ:(ct + 1) * NQ], pkv[C:128, :],
                        wtap[C:128, 1, 5:6])

                def emit_kv(upto):
                    while st["kv"] <= min(upto, NQT - 1):
                        ct = st["kv"]
                        pkv = ps2.tile([128, NQ], F32, name="pkv", tag="cv")
                        first = conv_matmuls(pkv, st_kv, ct, 128)
                        if False and st.get("qk_dep") is not None:
                            # keep the tile scheduler from hoisting kv-conv 1
                            # ahead of QK group 0 on the PE (its CoreSim DMA
                            # model is optimistic about the x chunk arrival)
                            add_dep_helper(first.ins, st["qk_dep"].ins,
                                           reason="kv conv after QK group")
                        nc.vector.tensor_copy(kT[:, ct * NQ:(ct + 1) * NQ],
                                              pkv[0:C, :])
                        emit_vt_add(pkv, ct)
                        st["kv"] += 1

                def emit_vnat(limit=NKC):
                    # v_nat transposes, batched 4 chunks per PSUM tile /
                    # copy to amortize the DVE PSUM-access overhead
                    top = st["kv"] * NQ
                    while st["vn"] < limit:
                        kc0 = st["vn"]
                        nb = min(4, NKC - kc0)
                        end = kc0 + nb - 1
                        cw_last = min(KC, N - end * KC)
                        if end * KC + cw_last > top:
                            break
                        tp = psC.tile([128, 4, C], BF16, name="tpv",
                                      tag="ctx")
                        for j in range(nb):
                            kc = kc0 + j
                            cw = min(KC, N - kc * KC)
                            nc.tensor.transpose(
                                tp[0:cw, j, :],
                                vT[C:128, kc * KC:kc * KC + cw],
                                ident_b[C:128, C:128])
                        cw = min(KC, N - (kc0 + nb - 1) * KC)
                        if nb == 4 and cw == KC:
                            nc.vector.tensor_copy(
                                v_nat[:, kc0:kc0 + nb, 0:C], tp[:, 0:nb, :])
                        else:
                            for j in range(nb):
                                kc = kc0 + j
                                cw = min(KC, N - kc * KC)
                                nc.vector.tensor_copy(
                                    v_nat[0:cw, kc, 0:C], tp[0:cw, j, :])
                        st["vn"] += nb

                def emit_qconv_slots(pq, qt, slots):
                    r0 = qt * 8
                    for i in slots:
                        s = SLOT_ORDER[i]
                        if s < 3:
                            rhs = xpT[:, r0:r0 + 8, s:s + W]
                            lh = st_q[:, :, s]
                        elif s == 3:
                            rhs = x3[:, r0 + 2:r0 + 10, 0:W]
                            lh = st_q[:, :, s]
                        else:
                            rhs = xpT[0:C, r0 + 2:r0 + 10, 2:2 + W]
                            lh = st_q[0:C, :, s]
                        nc.tensor.matmul(pq[:], lh, rhs,
                                         start=(i == 0), stop=(i == 4))

                def emit_qcopy(pq, qt):
                    if qt == 0:
                        # ACT is idle before the first exp; DVE is busy with
                        # lhsT builds
                        nc.scalar.add(qT[:, qt * NQ:(qt + 1) * NQ], pq[:],
                                      wtap[0:C, 0, 5:6])
                    else:
                        nc.vector.tensor_scalar_add(
                            qT[:, qt * NQ:(qt + 1) * NQ], pq[:], wtap[0:C, 0, 5:6])

                def emit_av_chunks(tiles, s, pctx, chunks):
                    # chunk-matmuls of one q-subtile into accumulator pctx
                    pT_tiles, dve_tiles = tiles
                    s0 = s * 128
                    sw = min(128, NQ - s0)
                    for kc in chunks:
                        cw = 64 if kc == NKC - 1 else 128
                        loc = CH_LOC[kc]
                        if loc[0] == "dve":
                            lhsT = dve_tiles[loc[1]][0:cw, s0:s0 + sw].bitcast(
                                mybir.dt.bfloat16)
                        else:
                            lhsT = pT_tiles[loc[1]][0:cw, loc[2], s0:s0 + sw]
                        nc.tensor.matmul(
                            pctx[0:sw, :],
                            lhsT,
                            v_nat[0:cw, kc, :],
                            start=(kc == 0), stop=(kc == NKC - 1))

                def emit_av_batch(pT_tiles, s, half):
                    # 25 chunk-matmuls of one q-subtile, split in two halves;
                    # one pending psum group at a time (zero-region rule)
                    if half == 0:
                        st["pctx"] = psC.tile([128, C + 1], F32,
                                              name="pctx", tag="ctx")
                    chunks = range(0, 13) if half == 0 else range(13, NKC)
                    emit_av_chunks(pT_tiles, s, st["pctx"], chunks)

                def emit_norm_sub(pctx, qt, s, on_act=False, swdge=False):
                    # normalize into the tile's gathered [128, 4, E] buffer;
                    # subs 1 and 3 each trigger a half-tile store DMA. The
                    # last tile runs half the muls on ACT (idle by then) and
                    # its second store through SWDGE so the two descriptor
                    # generations overlap.
                    s0 = s * 128
                    sw = min(128, NQ - s0)
                    inv = sbI.tile([128, 1], F32, name="inv", tag="inv")
                    nc.vector.reciprocal(inv[0:sw, :], pctx[0:sw, C:C + 1])
                    if s == 0:
                        st["osb4"] = sbO.tile([128, 4, E], F32, name="osb4",
                                              tag="out")
                    osb4 = st["osb4"]
                    if on_act:
                        nc.scalar.activation(
                            osb4[0:sw, s, :], pctx[0:sw, 0:C],
                            AF.Copy, scale=inv[0:sw, 0:1])
                    else:
                        nc.vector.tensor_scalar_mul(
                            osb4[0:sw, s, :], pctx[0:sw, 0:C], inv[0:sw, 0:1])
                    if s in (1, 3):
                        r0 = (s - 1) * 128
                        dma = nc.gpsimd.dma_start if swdge else nc.sync.dma_start
                        dma(out_d[qt, r0:r0 + 256, :].rearrange(
                                "(s p) e -> p s e", p=128),
                            osb4[:, s - 1:s + 1, :])

                def flush_prev(prev, g):
                    # AV batches one group later than minimal so the psC WAR
                    # (sub start vs previous norm) never stalls the PE
                    if prev is None or g < 1 or g > 8:
                        return
                    qt_prev, pT_tiles = prev
                    emit_av_batch(pT_tiles, (g - 1) // 2, (g - 1) % 2)
                    if (g - 1) % 2 == 1:
                        emit_norm_sub(st["pctx"], qt_prev, (g - 1) // 2)

                # ---- lead-in: q-conv(0) + kv(0) ----------------------------
                # PE order tuned for the kT critical path: xpT-only slots of
                # both convs first, then kv's x3 slot (so the kT copy can
                # start ASAP), then q's x3 slot + bias copy. vT add deferred
                # past QK g0 (engine-counter waits would otherwise put it on
                # QK's critical path).
                tap_sched = {0: (0,), 1: (1,), 2: (2,), 3: (3,), 4: (4,)}
                if level >= 2:
                    pq = ps2.tile([C, NQ], F32, name="pq", tag="cv")
                    emit_qconv_slots(pq, 0, range(4))
                    pkv0 = ps2.tile([128, NQ], F32, name="pkv", tag="cv")
                    conv_matmuls(pkv0, st_kv, 0, 128, slots=range(4))
                    conv_matmuls(pkv0, st_kv, 0, 128, slots=(4,))
                    # split so QK group 0 (k tokens 0-255) starts sooner
                    nc.vector.tensor_copy(kT[:, 0:256], pkv0[0:C, 0:256])
                    nc.vector.tensor_copy(kT[:, 256:NQ], pkv0[0:C, 256:NQ])
                    emit_qconv_slots(pq, 0, (4,))
                    emit_qcopy(pq, 0)
                    st["kv"] = 1
                    st["pkv0"] = pkv0

                prev = None
                last = NQT - 1
                for qt in range(NQT if level >= 5 else 0):
                    q0 = qt * NQ
                    pq_next = None
                    pT_tiles = []
                    dve_tiles = []
                    tiles = (pT_tiles, dve_tiles)
                    pc6 = [None, None, None, None]  # last-tile accumulators
                    if qt == last and level >= 6:
                        pc6[0] = ps2.tile([128, C + 1], F32, name="pc6a",
                                          tag="cv")
                        pc6[1] = ps2.tile([128, C + 1], F32, name="pc6b",
                                          tag="cv")
                    for g in range(NGRP):
                        pool = psSa if g % 2 == 0 else psSb
                        ps_s = pool.tile([128, 2, 512], F32, name="ps_s",
                                         tag="sa" if g % 2 == 0 else "sb")
                        for j, kc in enumerate(ACT_GRP[g]):
                            qk_mm = nc.tensor.matmul(
                                ps_s[:, j, 0:NQ],
                                kT[:, kc * KC:(kc + 1) * KC],
                                qT[:, q0:q0 + NQ],
                                start=True, stop=True)
                        if qt == 0:
                            # tile 0: kv convs + their kT/vT copies go FIRST
                            # so the dve sch (waiting on its QK) never
                            # head-of-line-blocks them on the DVE queue
                            if g == 0:
                                emit_vt_add(st["pkv0"], 0)
                            if g + 1 < NGRP:
                                emit_kv(KV_NEED[g + 1])
                        if g in DVE_AFTER:
                            # DVE-destined chunk: own 1-bank score lane +
                            # Schraudolph exp, fully decoupled from the ACT
                            # group tiles so the scheduler never chains them
                            kc = DVE_CH[DVE_AFTER[g]]
                            psd = psSd.tile([128, 512], F32, name="psd",
                                            tag="sd")
                            qk_mm = nc.tensor.matmul(
                                psd[:, 0:NQ],
                                kT[:, kc * KC:(kc + 1) * KC],
                                qT[:, q0:q0 + NQ],
                                start=True, stop=True)
                            pTd = sbPd.tile([128, NQ], mybir.dt.int16,
                                            name="pTd", tag="pd")
                            nc.vector.tensor_scalar(
                                pTd[:, :], psd[:, 0:NQ], SCH_A, SCH_B,
                                mybir.AluOpType.mult, mybir.AluOpType.add)
                            dve_tiles.append(pTd)
                        st["qk_dep"] = qk_mm
                        flush_prev(prev, g)
                        if qt == last and level >= 6:
                            if g == 8:
                                # tile-5 AV fully flushed; psC free for sub 3
                                pc6[3] = psC.tile([128, C + 1], F32,
                                                  name="pc6d", tag="ctx")
                            if g >= 1:
                                emit_av_chunks(tiles, 0, pc6[0], STEP[g - 1])
                                emit_av_chunks(tiles, 1, pc6[1], STEP[g - 1])
                            if g == NGRP - 1:
                                # catch-up: subs 2/3 over steps 0..8
                                hi = min(STEP[g])
                                emit_av_chunks(tiles, 2, pc6[2], range(0, hi))
                                emit_av_chunks(tiles, 3, pc6[3], range(0, hi))
                        if qt == 0:
                            # v_nat transposes wait for tile 0's PE
                            # congestion to pass (not needed until tile 1's
                            # AV batches)
                            if g == NGRP - 1 and qt + 1 < NQT:
                                emit_vnat(16)
                                pq_next = ps2.tile([C, NQ], F32,
                                                   name="pq", tag="cv")
                                emit_qconv_slots(pq_next, 1, range(5))
                                emit_qcopy(pq_next, 1)
                        elif qt + 1 < NQT:
                            if qt == 1 and g == 0:
                                emit_vnat()
                            if g in tap_sched:
                                if pq_next is None:
                                    pq_next = ps2.tile([C, NQ], F32,
                                                       name="pq", tag="cv")
                                emit_qconv_slots(pq_next, qt + 1, tap_sched[g])
                            if g == 5:
                                emit_qcopy(pq_next, qt + 1)
                        ng = len(ACT_GRP[g])
                        pTt = sbP.tile([128, 2, NQ], BF16, name="pTt", tag="p")
                        nc.scalar.activation(
                            pTt[:, 0:ng, :], ps_s[:, 0:ng, 0:NQ],
                            AF.Exp, scale=8.0)
                        pT_tiles.append(pTt)
                        if qt == last and level >= 6 and g == 8:
                            # psSa's last score group read; slot free for sub 2
                            pc6[2] = psSa.tile([128, C + 1], F32,
                                               name="pc6c", tag="sa")
                    if level >= 6:
                        if qt < last:
                            prev = (qt, tiles)
                        else:
                            # tail: per sub, last step's chunks then norm,
                            # so the half-tile stores fire ASAP
                            for s in range(4):
                                emit_av_chunks(tiles, s, pc6[s],
                                               STEP[NGRP - 1])
                                # the earlier-ready half rides the slower
                                # SWDGE gen; the later half takes HWDGE
                                emit_norm_sub(pc6[s], qt, s,
                                              on_act=(s in (1, 3)),
                                              swdge=(s == 1))

    nc.compile()
    return nc


def _get_nc():
    if "nc" not in _CACHE:
        _CACHE["nc"] = _build()
    return _CACHE["nc"]


def kernel(x, wq, bq, wk, bk, wv, bv, Wp, bp):
    from concourse.bass_utils import run_bass_kernel_spmd

    nc = _get_nc()
    x = np.asarray(x, dtype=np.float32)
    wtap, stv = _prep_weights(wq, wk, wv, bq, bv, Wp, bp)
    shared = {"wtap": wtap, "stv": stv}
    in_maps = []
    for i in range(NCORES):
        xp, x3 = _prep_x(x[i])
        in_maps.append(dict(shared, x=xp, x3=x3))
    res = run_bass_kernel_spmd(nc, in_maps, core_ids=list(range(NCORES)))
    out = np.stack([
        np.asarray(res.results[i]["out"])
        .reshape(NQT, 4 * 128, E)[:, 0:NQ, :].reshape(H, W, E)
        for i in range(NCORES)
    ])
    return out


# revision 66
# speedup vs baseline: 1.0040x; 1.0040x over previous
"""ConvAttention Trainium2 kernel (v7, ~83us vs 101us v5 baseline).

Per-core (data-parallel over batch, 8 cores, 1 image each):
  q/k/v = depthwise 3x3 conv over x [56,56,64], then full attention over
  N=3136 tokens with softmax(q.k * 8), then ctx @ Wp + bp.

Layout:
  - x staged on host as two [128, 58, 58] images (xpT row-shift pair, x3
    col-shift pair) in f32r; convs are tap-stacked K=128 matmuls (5 per
    conv tile); Wp/bv/bp fold into the v-conv; bk dropped (cancels).
  - The diag conv lhsT blocks are built ON-DEVICE (affine_select mask x
    per-partition tap scalars) so the serial-DMA front carries only 48B
    of taps + the dense v-projection blocks before the x row chunks.
  - exp is split across engines: 20 k-chunks/tile on ACT (10 groups of 2
    double-buffered in psSa/psSb), 5 chunks on DVE as a Schraudolph bf16
    exp (i16 = rint(8s*A + B) bitcast to bf16, ~3% elem error, scattered
    chunks so the softmax averages it out; measured 9.1e-3 final err).
    The DVE chunks flow through a dedicated 1-bank psSd score lane +
    separate pTd tiles because the tile scheduler linearizes all
    consumers of a shared PSUM tile with cross-engine semaphores.
  - AV uses pT chunks as lhsT into [token, embed] accumulators with a
    ones column for the softmax denominator (reciprocal + mul norm).
  - Last q-tile's AV interleaves under its own exp groups (subs 0/1
    chunk-wise in the idle conv-psum slots, subs 2/3 catch up in the
    freed psSa/psC slots), and per-tile stores go out as two gathered
    half-tile DMAs, so only ~4us of tail remains after the final exp.
  - PE p-state warmup: pe_busy_start is sticky, so two early matmuls on
    a DVE-memset tile start the clock ramp during the DMA wait.
"""

import sys

import numpy as np

if "/opt/trn_rl_repo" not in sys.path:
    sys.path.insert(0, "/opt/trn_rl_repo")

H = 56
W = 56
C = 64
E = 64
N = H * W               # 3136 tokens
HP = H + 2              # padded
WP = W + 2
NQ = 448                # q-tile (8 spatial rows)
NQT = N // NQ           # 7
KC = 128                # k-chunk (partition dim of s^T tiles)
NKC = (N + KC - 1) // KC  # 25 (last chunk is 64 real tokens)
NPAD = NKC * KC         # 3200 (k padded with zeros)
NCORES = 8

# exp chunk assignment per tile: 20 chunks on ACT in 10 double-buffered
# groups of 2 (psSa/psSb, 2 banks each), 5 chunks on DVE via Schraudolph
# from a dedicated 1-bank score lane (psSd) so the scheduler never chains
# the ACT exp behind the DVE read of a shared tile
DVE_CH = (4, 9, 14, 19, 24)
ACT_GRP = [[0, 1], [2, 3], [5, 6], [7, 8], [10, 11],
           [12, 13], [15, 16], [17, 18], [20, 21], [22, 23]]
NGRP = len(ACT_GRP)
# dve chunk emitted right after this group's QK
DVE_AFTER = {1: 0, 3: 1, 5: 2, 7: 3, 9: 4}
# chunks covered by emission step g (group + trailing dve chunk)
STEP = [ACT_GRP[g] + ([DVE_CH[DVE_AFTER[g]]] if g in DVE_AFTER else [])
        for g in range(NGRP)]
# chunk -> ("act", g, j) or ("dve", i)
CH_LOC = {}
for _g, _js in enumerate(ACT_GRP):
    for _j, _kc in enumerate(_js):
        CH_LOC[_kc] = ("act", _g, _j)
for _i, _kc in enumerate(DVE_CH):
    CH_LOC[_kc] = ("dve", _i)
# kv-conv tile that must be complete before QK of step g (any q-tile)
KV_NEED = [min(((max(STEP[g]) + 1) * KC - 1) // NQ, NQT - 1)
           for g in range(NGRP)]
# stacked conv slots: (lower tap, upper tap or None); taps t = 3*i + j.
# Slots 0-2 pair rows 0+1 via xpT's row-shifted upper half; slot 3 pairs
# (2,0)+(2,1) via x3's col-shifted upper half; slot 4 is the single (2,2).
CONV_SLOTS = [(0, 3), (1, 4), (2, 5), (6, 7), (8, None)]
# emission order: the single K=64 tap first (fewest dependencies)
SLOT_ORDER = (4, 0, 1, 2, 3)

# Schraudolph bf16 exp constants: i16 = trunc((8*s)*A + B) bitcast to
# bf16 approximates exp(8*s) with ~3% element error; the scattered
# DVE_CH pattern averages it out in the softmax (measured ~9.6e-3 final)
SCH_A = 8.0 * 128.0 / 0.6931471805599453
SCH_B = 127.0 * 128.0 - 5.0

_CACHE = {}


def _prep_x(xi):
    """Host staging: [56,56,64] -> (xpT, x3) [128, HP, WP] float32."""
    base = np.zeros((C, HP, WP), np.float32)
    base[:, 1:1 + H, 1:1 + W] = np.ascontiguousarray(xi.transpose(2, 0, 1))
    xp = np.zeros((128, HP, WP), np.float32)
    xp[0:C] = base
    xp[C:128, 0:HP - 1] = base[:, 1:HP]
    x3 = np.zeros((128, HP, WP), np.float32)
    x3[0:C] = base
    x3[C:128, :, 0:WP - 1] = base[:, :, 1:WP]
    return xp, x3


def _prep_weights(wq, wk, wv, bq, bv, Wp, bp):
    """Host staging of the conv weights (diag blocks built on-device).

    wtap [128, 2, 5]: per-partition tap values, [:,0,s]=wq, [:,1,s]=wk
      (partition p<64: lower tap of slot s for channel p; p>=64: upper tap).
    stv [128, 64, 5]: the dense v-projection blocks wv[tap]*Wp, laid out
      [p, out_col, slot] to sit contiguously behind the on-device diag.
    bqb1 [128, 2]: col 0 rows 0-63 = bq; col 1 rows 64-127 = bv@Wp + bp.
    """
    wq = np.asarray(wq, np.float32).reshape(9, C)
    wk = np.asarray(wk, np.float32).reshape(9, C)
    wv = np.asarray(wv, np.float32).reshape(9, C)
    Wp = np.asarray(Wp, np.float32)
    wtap = np.zeros((128, 2, 6), np.float32)
    stv = np.zeros((128, C, 5), np.float32)
    for s, (lt, ut) in enumerate(CONV_SLOTS):
        wtap[0:C, 0, s] = wq[lt]
        wtap[0:C, 1, s] = wk[lt]
        stv[0:C, :, s] = wv[lt][:, None] * Wp
        if ut is not None:
            wtap[C:128, 0, s] = wq[ut]
            wtap[C:128, 1, s] = wk[ut]
            stv[C:128, :, s] = wv[ut][:, None] * Wp
    # biases ride along in wtap[:, :, 5]: [0:C, 0, 5] = bq,
    # [C:128, 1, 5] = bv@Wp + bp
    wtap[0:C, 0, 5] = np.asarray(bq, np.float32)
    wtap[C:128, 1, 5] = (np.asarray(bv, np.float32) @ Wp
                         + np.asarray(bp, np.float32))
    return wtap, stv


def _build(level=99):
    import concourse.bacc as bacc
    import concourse.tile as tile
    from concourse import mybir
    from concourse.masks import make_identity
    from concourse.tile import add_dep_helper

    F32 = mybir.dt.float32
    F32R = mybir.dt.float32r
    BF16 = mybir.dt.bfloat16
    AF = mybir.ActivationFunctionType

    nc = bacc.Bacc(None, target_bir_lowering=False, debug=False)

    x_d = nc.dram_tensor("x", [128, HP, WP], F32R, kind="ExternalInput")
    x3_d = nc.dram_tensor("x3", [128, HP, WP], F32R, kind="ExternalInput")
    wtap_d = nc.dram_tensor("wtap", [128, 2, 6], F32, kind="ExternalInput")
    stv_d = nc.dram_tensor("stv", [128, C, 5], F32R, kind="ExternalInput")
    # per-tile padded output: token qt*448 + s*128 + p -> out_d[qt, s*128+p]
    # (s=3 rows 64-127 are never written; host slices them off)
    out_d = nc.dram_tensor("out", [NQT, 4 * 128, E], F32, kind="ExternalOutput")

    # row-chunked image loads, sized for the serial DMA model: rows 0-9 for
    # tile 0's convs up front, then uniform 8-row chunks just ahead of the
    # kv-conv demand curve. x3 rows 0-1 are never read (conv slot 3 reads
    # rows 8ct+2..8ct+9), so its first chunk starts at row 2.
    RCH = [10, 18, 26, 34, 42, 50, HP]

    with tile.TileContext(nc) as tc:
        with tc.tile_pool(name="const", bufs=1) as const, \
             tc.tile_pool(name="big", bufs=1) as big:
            # warmup tile first: DVE memset -> PE p-state ramp starts ~1us in
            # (memset of f32r is not a valid ISA op; memset as f32, matmul
            # reads it through an f32r bitcast)
            wtile = const.tile([128, 128], F32)
            nc.vector.memset(wtile[:], 0.0)

            xpT = big.tile([128, HP, WP], F32R)
            x3 = big.tile([128, HP, WP], F32R)
            wtap = const.tile([128, 2, 6], F32)
            # [p, out_col, slot]: cols 0-63 diag (on-device), v block via DMA
            st_q = const.tile([128, C, 5], F32R)
            st_kv = const.tile([128, 128, 5], F32R)

            # DMA plan: single (sync) queue; the shared DMA engine device is
            # serial in the cost model and HWDGE descriptor generation costs
            # 625ns per dma_start, so both order and COUNT of front DMAs
            # matter. The diag lhsT blocks are built on-device (wtap is 48B
            # vs 3.8KB staged) and the biases ride in wtap, keeping the
            # serial front at 4 descriptors before the row chunks.
            nc.sync.dma_start(wtap[:], wtap_d[:])
            nc.sync.dma_start(xpT[:, 0:RCH[0], :], x_d[:, 0:RCH[0], :])
            nc.sync.dma_start(st_kv[:, C:128, :], stv_d[:])
            nc.sync.dma_start(x3[:, 2:RCH[0], :], x3_d[:, 2:RCH[0], :])
            for ci in range(len(RCH) - 1):
                r0, r1 = RCH[ci], RCH[ci + 1]
                nc.sync.dma_start(xpT[:, r0:r1, :], x_d[:, r0:r1, :])
                nc.sync.dma_start(x3[:, r0:r1, :], x3_d[:, r0:r1, :])

            ident_f = const.tile([128, 128], F32)
            make_identity(nc, ident_f[:])
            ident_b = const.tile([128, 128], BF16)
            nc.vector.tensor_copy(ident_b[:], ident_f[:])

            # two-half diagonal mask M[p, y] = (y == p mod 64) for the
            # on-device diag builds
            dmask = const.tile([128, C], F32)
            nc.gpsimd.memset(dmask[:], 0.0)
            for half in (0, 1):
                nc.gpsimd.affine_select(
                    out=dmask[half * C:(half + 1) * C, :],
                    in_=dmask[half * C:(half + 1) * C, :],
                    compare_op=mybir.AluOpType.not_equal,
                    fill=1.0,
                    base=0,
                    pattern=[[-1, C]],
                    channel_multiplier=1,
                )
            # st_q / st_kv diag blocks: M * per-partition tap value
            for s in range(5):
                nc.vector.tensor_scalar_mul(
                    st_q[:, :, s], dmask[:], wtap[:, 0, s:s + 1])
                nc.vector.tensor_scalar_mul(
                    st_kv[:, 0:C, s], dmask[:], wtap[:, 1, s:s + 1])

            zsc = const.tile([128, 128], F32)
            nc.vector.memset(zsc[:], 0.0)
            ones_f = const.tile([128, NKC], F32)
            nc.vector.memset(ones_f[:], 1.0)

            qT = big.tile([C, N], F32R)            # q^T  [c, token]
            kT = big.tile([C, NPAD], F32R)         # k^T  [c, token], zero pad
            vT = big.tile([128, N], BF16)          # v''^T on partitions 64-127
            v_nat = big.tile([128, NKC, C + 1], BF16)  # [tok%128, chunk, e|1]

            nc.vector.tensor_copy(kT[:, N:NPAD], zsc[0:C, 0:NPAD - N])
            nc.vector.tensor_copy(v_nat[:, :, C], ones_f[:])

            with tc.tile_pool(name="ps2", bufs=2, space="PSUM") as ps2, \
                 tc.tile_pool(name="psSa", bufs=1, space="PSUM") as psSa, \
                 tc.tile_pool(name="psSb", bufs=1, space="PSUM") as psSb, \
                 tc.tile_pool(name="psSd", bufs=1, space="PSUM") as psSd, \
                 tc.tile_pool(name="psC", bufs=1, space="PSUM") as psC, \
                 tc.tile_pool(name="sbP", bufs=2 * NGRP) as sbP, \
                 tc.tile_pool(name="sbPd", bufs=2 * len(DVE_CH)) as sbPd, \
                 tc.tile_pool(name="sbO", bufs=4) as sbO, \
                 tc.tile_pool(name="sbI", bufs=4) as sbI:

                # PE warmup: set pe_busy_start early so the clock ramps to
                # full rate during the image DMA wait
                # pe_busy_start is sticky from the first matmul (no reset on
                # idle), so two early matmuls are enough to start the clock
                # ramp; the PE hits full rate ~3us later regardless of idling
                ptw = ps2.tile([128, NQ], F32, name="ptw", tag="cv")
                for _ in range(2):
                    nc.tensor.matmul(ptw[0:128, 0:128],
                                     wtile[:, 0:128].bitcast(F32R),
                                     wtile[:, 0:128].bitcast(F32R),
                                     start=True, stop=True)

                # ---- incremental emitters ----------------------------------
                st = {"kv": 0, "vn": 0, "pctx": None}

                def conv_matmuls(pdst, lhsT, ct, mwid, slots=range(5)):
                    r0 = ct * 8
                    first = None
                    for i in slots:
                        s = SLOT_ORDER[i]
                        if s < 3:
                            rhs = xpT[:, r0:r0 + 8, s:s + W]
                            lh = lhsT[:, 0:mwid, s]
                        elif s == 3:
                            rhs = x3[:, r0 + 2:r0 + 10, 0:W]
                            lh = lhsT[:, 0:mwid, s]
                        else:
                            rhs = xpT[0:C, r0 + 2:r0 + 10, 2:2 + W]
                            lh = lhsT[0:C, 0:mwid, s]
                        mm = nc.tensor.matmul(pdst[:], lh, rhs,
                                              start=(i == 0), stop=(i == 4))
                        if first is None:
                            first = mm
                    return first

                def emit_vt_add(pkv, ct):
                    nc.vector.tensor_scalar_add(
                        vT[C:128, ct * NQ:(ct + 1) * NQ], pkv[C:128, :],
                        wtap[C:128, 1, 5:6])

                def emit_kv(upto):
                    while st["kv"] <= min(upto, NQT - 1):
                        ct = st["kv"]
                        pkv = ps2.tile([128, NQ], F32, name="pkv", tag="cv")
                        first = conv_matmuls(pkv, st_kv, ct, 128)
                        if False and st.get("qk_dep") is not None:
                            # keep the tile scheduler from hoisting kv-conv 1
                            # ahead of QK group 0 on the PE (its CoreSim DMA
                            # model is optimistic about the x chunk arrival)
                            add_dep_helper(first.ins, st["qk_dep"].ins,
                                           reason="kv conv after QK group")
                        nc.vector.tensor_copy(kT[:, ct * NQ:(ct + 1) * NQ],
                                              pkv[0:C, :])
                        emit_vt_add(pkv, ct)
                        st["kv"] += 1

                def emit_vnat(limit=NKC):
                    # v_nat transposes, batched 4 chunks per PSUM tile /
                    # copy to amortize the DVE PSUM-access overhead
                    top = st["kv"] * NQ
                    while st["vn"] < limit:
                        kc0 = st["vn"]
                        nb = min(4, NKC - kc0)
                        end = kc0 + nb - 1
                        cw_last = min(KC, N - end * KC)
                        if end * KC + cw_last > top:
                            break
                        tp = psC.tile([128, 4, C], BF16, name="tpv",
                                      tag="ctx")
                        for j in range(nb):
                            kc = kc0 + j
                            cw = min(KC, N - kc * KC)
                            nc.tensor.transpose(
                                tp[0:cw, j, :],
                                vT[C:128, kc * KC:kc * KC + cw],
                                ident_b[C:128, C:128])
                        cw = min(KC, N - (kc0 + nb - 1) * KC)
                        if nb == 4 and cw == KC:
                            nc.vector.tensor_copy(
                                v_nat[:, kc0:kc0 + nb, 0:C], tp[:, 0:nb, :])
                        else:
                            for j in range(nb):
                                kc = kc0 + j
                                cw = min(KC, N - kc * KC)
                                nc.vector.tensor_copy(
                                    v_nat[0:cw, kc, 0:C], tp[0:cw, j, :])
                        st["vn"] += nb

                def emit_qconv_slots(pq, qt, slots):
                    r0 = qt * 8
                    for i in slots:
                        s = SLOT_ORDER[i]
                        if s < 3:
                            rhs = xpT[:, r0:r0 + 8, s:s + W]
                            lh = st_q[:, :, s]
                        elif s == 3:
                            rhs = x3[:, r0 + 2:r0 + 10, 0:W]
                            lh = st_q[:, :, s]
                        else:
                            rhs = xpT[0:C, r0 + 2:r0 + 10, 2:2 + W]
                            lh = st_q[0:C, :, s]
                        nc.tensor.matmul(pq[:], lh, rhs,
                                         start=(i == 0), stop=(i == 4))

                def emit_qcopy(pq, qt):
                    if qt == 0:
                        # ACT is idle before the first exp; DVE is busy with
                        # lhsT builds
                        nc.scalar.add(qT[:, qt * NQ:(qt + 1) * NQ], pq[:],
                                      wtap[0:C, 0, 5:6])
                    else:
                        nc.vector.tensor_scalar_add(
                            qT[:, qt * NQ:(qt + 1) * NQ], pq[:], wtap[0:C, 0, 5:6])

                def emit_av_chunks(tiles, s, pctx, chunks):
                    # chunk-matmuls of one q-subtile into accumulator pctx
                    pT_tiles, dve_tiles = tiles
                    s0 = s * 128
                    sw = min(128, NQ - s0)
                    for kc in chunks:
                        cw = 64 if kc == NKC - 1 else 128
                        loc = CH_LOC[kc]
                        if loc[0] == "dve":
                            lhsT = dve_tiles[loc[1]][0:cw, s0:s0 + sw].bitcast(
                                mybir.dt.bfloat16)
                        else:
                            lhsT = pT_tiles[loc[1]][0:cw, loc[2], s0:s0 + sw]
                        nc.tensor.matmul(
                            pctx[0:sw, :],
                            lhsT,
                            v_nat[0:cw, kc, :],
                            start=(kc == 0), stop=(kc == NKC - 1))

                def emit_av_batch(pT_tiles, s, half):
                    # 25 chunk-matmuls of one q-subtile, split in two halves;
                    # one pending psum group at a time (zero-region rule)
                    if half == 0:
                        st["pctx"] = psC.tile([128, C + 1], F32,
                                              name="pctx", tag="ctx")
                    chunks = range(0, 13) if half == 0 else range(13, NKC)
                    emit_av_chunks(pT_tiles, s, st["pctx"], chunks)

                def emit_norm_sub(pctx, qt, s, on_act=False, swdge=False):
                    # normalize into the tile's gathered [128, 4, E] buffer;
                    # subs 1 and 3 each trigger a half-tile store DMA. The
                    # last tile runs half the muls on ACT (idle by then) and
                    # its second store through SWDGE so the two descriptor
                    # generations overlap.
                    s0 = s * 128
                    sw = min(128, NQ - s0)
                    inv = sbI.tile([128, 1], F32, name="inv", tag="inv")
                    nc.vector.reciprocal(inv[0:sw, :], pctx[0:sw, C:C + 1])
                    if s == 0:
                        st["osb4"] = sbO.tile([128, 4, E], F32, name="osb4",
                                              tag="out")
                    osb4 = st["osb4"]
                    if on_act:
                        nc.scalar.activation(
                            osb4[0:sw, s, :], pctx[0:sw, 0:C],
                            AF.Copy, scale=inv[0:sw, 0:1])
                    else:
                        nc.vector.tensor_scalar_mul(
                            osb4[0:sw, s, :], pctx[0:sw, 0:C], inv[0:sw, 0:1])
                    if s in (1, 3):
                        r0 = (s - 1) * 128
                        dma = nc.gpsimd.dma_start if swdge else nc.sync.dma_start
                        dma(out_d[qt, r0:r0 + 256, :].rearrange(
                                "(s p) e -> p s e", p=128),
                            osb4[:, s - 1:s + 1, :])

                def flush_prev(prev, g):
                    # AV batches one group later than minimal so the psC WAR
                    # (sub start vs previous norm) never stalls the PE
                    if prev is None or g < 1 or g > 8:
                        return
                    qt_prev, pT_tiles = prev
                    emit_av_batch(pT_tiles, (g - 1) // 2, (g - 1) % 2)
                    if (g - 1) % 2 == 1:
                        emit_norm_sub(st["pctx"], qt_prev, (g - 1) // 2)

                # ---- lead-in: q-conv(0) + kv(0) ----------------------------
                # PE order tuned for the kT critical path: xpT-only slots of
                # both convs first, then kv's x3 slot (so the kT copy can
                # start ASAP), then q's x3 slot + bias copy. vT add deferred
                # past QK g0 (engine-counter waits would otherwise put it on
                # QK's critical path).
                tap_sched = {0: (0,), 1: (1,), 2: (2,), 3: (3,), 4: (4,)}
                if level >= 2:
                    pq = ps2.tile([C, NQ], F32, name="pq", tag="cv")
                    emit_qconv_slots(pq, 0, range(4))
                    pkv0 = ps2.tile([128, NQ], F32, name="pkv", tag="cv")
                    conv_matmuls(pkv0, st_kv, 0, 128, slots=range(4))
                    conv_matmuls(pkv0, st_kv, 0, 128, slots=(4,))
                    nc.vector.tensor_copy(kT[:, 0:NQ], pkv0[0:C, :])
                    emit_qconv_slots(pq, 0, (4,))
                    emit_qcopy(pq, 0)
                    st["kv"] = 1
                    st["pkv0"] = pkv0

                prev = None
                last = NQT - 1
                for qt in range(NQT if level >= 5 else 0):
                    q0 = qt * NQ
                    pq_next = None
                    pT_tiles = []
                    dve_tiles = []
                    tiles = (pT_tiles, dve_tiles)
                    pc6 = [None, None, None, None]  # last-tile accumulators
                    if qt == last and level >= 6:
                        pc6[0] = ps2.tile([128, C + 1], F32, name="pc6a",
                                          tag="cv")
                        pc6[1] = ps2.tile([128, C + 1], F32, name="pc6b",
                                          tag="cv")
                    for g in range(NGRP):
                        pool = psSa if g % 2 == 0 else psSb
                        ps_s = pool.tile([128, 2, 512], F32, name="ps_s",
                                         tag="sa" if g % 2 == 0 else "sb")
                        for j, kc in enumerate(ACT_GRP[g]):
                            qk_mm = nc.tensor.matmul(
                                ps_s[:, j, 0:NQ],
                                kT[:, kc * KC:(kc + 1) * KC],
                                qT[:, q0:q0 + NQ],
                                start=True, stop=True)
                        if qt == 0:
                            # tile 0: kv convs + their kT/vT copies go FIRST
                            # so the dve sch (waiting on its QK) never
                            # head-of-line-blocks them on the DVE queue
                            if g == 0:
                                emit_vt_add(st["pkv0"], 0)
                            if g + 1 < NGRP:
                                emit_kv(KV_NEED[g + 1])
                        if g in DVE_AFTER:
                            # DVE-destined chunk: own 1-bank score lane +
                            # Schraudolph exp, fully decoupled from the ACT
                            # group tiles so the scheduler never chains them
                            kc = DVE_CH[DVE_AFTER[g]]
                            psd = psSd.tile([128, 512], F32, name="psd",
                                            tag="sd")
                            qk_mm = nc.tensor.matmul(
                                psd[:, 0:NQ],
                                kT[:, kc * KC:(kc + 1) * KC],
                                qT[:, q0:q0 + NQ],
                                start=True, stop=True)
                            pTd = sbPd.tile([128, NQ], mybir.dt.int16,
                                            name="pTd", tag="pd")
                            nc.vector.tensor_scalar(
                                pTd[:, :], psd[:, 0:NQ], SCH_A, SCH_B,
                                mybir.AluOpType.mult, mybir.AluOpType.add)
                            dve_tiles.append(pTd)
                        st["qk_dep"] = qk_mm
                        flush_prev(prev, g)
                        if qt == last and level >= 6:
                            if g == 8:
                                # tile-5 AV fully flushed; psC free for sub 3
                                pc6[3] = psC.tile([128, C + 1], F32,
                                                  name="pc6d", tag="ctx")
                            if g >= 1:
                                emit_av_chunks(tiles, 0, pc6[0], STEP[g - 1])
                                emit_av_chunks(tiles, 1, pc6[1], STEP[g - 1])
                            if g == NGRP - 1:
                                # catch-up: subs 2/3 over steps 0..8
                                hi = min(STEP[g])
                                emit_av_chunks(tiles, 2, pc6[2], range(0, hi))
                                emit_av_chunks(tiles, 3, pc6[3], range(0, hi))
                        if qt == 0:
                            # v_nat transposes wait for tile 0's PE
                            # congestion to pass (not needed until tile 1's
                            # AV batches)
                            if g == NGRP - 1 and qt + 1 < NQT:
                                emit_vnat(16)
                                pq_next = ps2.tile([C, NQ], F32,
                                                   name="pq", tag="cv")
                                emit_qconv_slots(pq_next, 1, range(5))
                                emit_qcopy(pq_next, 1)
                        elif qt + 1 < NQT:
                            if qt == 1 and g == 0:
                                emit_vnat()
                            if g in tap_sched:
                                if pq_next is None:
                                    pq_next = ps2.tile([C, NQ], F32,
                                                       name="pq", tag="cv")
                                emit_qconv_slots(pq_next, qt + 1, tap_sched[g])
                            if g == 5:
                                emit_qcopy(pq_next, qt + 1)
                        ng = len(ACT_GRP[g])
                        pTt = sbP.tile([128, 2, NQ], BF16, name="pTt", tag="p")
                        nc.scalar.activation(
                            pTt[:, 0:ng, :], ps_s[:, 0:ng, 0:NQ],
                            AF.Exp, scale=8.0)
                        pT_tiles.append(pTt)
                        if qt == last and level >= 6 and g == 8:
                            # psSa's last score group read; slot free for sub 2
                            pc6[2] = psSa.tile([128, C + 1], F32,
                                               name="pc6c", tag="sa")
                    if level >= 6:
                        if qt < last:
                            prev = (qt, tiles)
                        else:
                            # tail: per sub, last step's chunks then norm,
                            # so the half-tile stores fire ASAP
                            for s in range(4):
                                emit_av_chunks(tiles, s, pc6[s],
                                               STEP[NGRP - 1])
                                # the earlier-ready half rides the slower
                                # SWDGE gen; the later half takes HWDGE
                                emit_norm_sub(pc6[s], qt, s,
                                              on_act=(s in (1, 3)),
                                              swdge=(s == 1))

    nc.compile()
    return nc


def _get_nc():
    if "nc" not in _CACHE:
        _CACHE["nc"] = _build()
    return _CACHE["nc"]


def kernel(x, wq, bq, wk, bk, wv, bv, Wp, bp):
    from concourse.bass_utils import run_bass_kernel_spmd

    nc = _get_nc()
    x = np.asarray(x, dtype=np.float32)
    wtap, stv = _prep_weights(wq, wk, wv, bq, bv, Wp, bp)
    shared = {"wtap": wtap, "stv": stv}
    in_maps = []
    for i in range(NCORES):
        xp, x3 = _prep_x(x[i])
        in_maps.append(dict(shared, x=xp, x3=x3))
    res = run_bass_kernel_spmd(nc, in_maps, core_ids=list(range(NCORES)))
    out = np.stack([
        np.asarray(res.results[i]["out"])
        .reshape(NQT, 4 * 128, E)[:, 0:NQ, :].reshape(H, W, E)
        for i in range(NCORES)
    ])
    return out


# revision 67
# speedup vs baseline: 1.0046x; 1.0006x over previous
"""ConvAttention Trainium2 kernel (v7, ~83us vs 101us v5 baseline).

Per-core (data-parallel over batch, 8 cores, 1 image each):
  q/k/v = depthwise 3x3 conv over x [56,56,64], then full attention over
  N=3136 tokens with softmax(q.k * 8), then ctx @ Wp + bp.

Layout:
  - x staged on host as two [128, 58, 58] images (xpT row-shift pair, x3
    col-shift pair) in f32r; convs are tap-stacked K=128 matmuls (5 per
    conv tile); Wp/bv/bp fold into the v-conv; bk dropped (cancels).
  - The diag conv lhsT blocks are built ON-DEVICE (affine_select mask x
    per-partition tap scalars) so the serial-DMA front carries only 48B
    of taps + the dense v-projection blocks before the x row chunks.
  - exp is split across engines: 20 k-chunks/tile on ACT (10 groups of 2
    double-buffered in psSa/psSb), 5 chunks on DVE as a Schraudolph bf16
    exp (i16 = rint(8s*A + B) bitcast to bf16, ~3% elem error, scattered
    chunks so the softmax averages it out; measured 9.1e-3 final err).
    The DVE chunks flow through a dedicated 1-bank psSd score lane +
    separate pTd tiles because the tile scheduler linearizes all
    consumers of a shared PSUM tile with cross-engine semaphores.
  - AV uses pT chunks as lhsT into [token, embed] accumulators with a
    ones column for the softmax denominator (reciprocal + mul norm).
  - Last q-tile's AV interleaves under its own exp groups (subs 0/1
    chunk-wise in the idle conv-psum slots, subs 2/3 catch up in the
    freed psSa/psC slots), and per-tile stores go out as two gathered
    half-tile DMAs, so only ~4us of tail remains after the final exp.
  - PE p-state warmup: pe_busy_start is sticky, so two early matmuls on
    a DVE-memset tile start the clock ramp during the DMA wait.
"""

import sys

import numpy as np

if "/opt/trn_rl_repo" not in sys.path:
    sys.path.insert(0, "/opt/trn_rl_repo")

H = 56
W = 56
C = 64
E = 64
N = H * W               # 3136 tokens
HP = H + 2              # padded
WP = W + 2
NQ = 448                # q-tile (8 spatial rows)
NQT = N // NQ           # 7
KC = 128                # k-chunk (partition dim of s^T tiles)
NKC = (N + KC - 1) // KC  # 25 (last chunk is 64 real tokens)
NPAD = NKC * KC         # 3200 (k padded with zeros)
NCORES = 8

# exp chunk assignment per tile: 20 chunks on ACT in 10 double-buffered
# groups of 2 (psSa/psSb, 2 banks each), 5 chunks on DVE via Schraudolph
# from a dedicated 1-bank score lane (psSd) so the scheduler never chains
# the ACT exp behind the DVE read of a shared tile
DVE_CH = (4, 9, 14, 19, 24)
ACT_GRP = [[0, 1], [2, 3], [5, 6], [7, 8], [10, 11],
           [12, 13], [15, 16], [17, 18], [20, 21], [22, 23]]
NGRP = len(ACT_GRP)
# dve chunk emitted right after this group's QK
DVE_AFTER = {1: 0, 3: 1, 5: 2, 7: 3, 9: 4}
# chunks covered by emission step g (group + trailing dve chunk)
STEP = [ACT_GRP[g] + ([DVE_CH[DVE_AFTER[g]]] if g in DVE_AFTER else [])
        for g in range(NGRP)]
# chunk -> ("act", g, j) or ("dve", i)
CH_LOC = {}
for _g, _js in enumerate(ACT_GRP):
    for _j, _kc in enumerate(_js):
        CH_LOC[_kc] = ("act", _g, _j)
for _i, _kc in enumerate(DVE_CH):
    CH_LOC[_kc] = ("dve", _i)
# kv-conv tile that must be complete before QK of step g (any q-tile)
KV_NEED = [min(((max(STEP[g]) + 1) * KC - 1) // NQ, NQT - 1)
           for g in range(NGRP)]
# stacked conv slots: (lower tap, upper tap or None); taps t = 3*i + j.
# Slots 0-2 pair rows 0+1 via xpT's row-shifted upper half; slot 3 pairs
# (2,0)+(2,1) via x3's col-shifted upper half; slot 4 is the single (2,2).
CONV_SLOTS = [(0, 3), (1, 4), (2, 5), (6, 7), (8, None)]
# emission order: the single K=64 tap first (fewest dependencies)
SLOT_ORDER = (4, 0, 1, 2, 3)

# Schraudolph bf16 exp constants: i16 = trunc((8*s)*A + B) bitcast to
# bf16 approximates exp(8*s) with ~3% element error; the scattered
# DVE_CH pattern averages it out in the softmax (measured ~9.6e-3 final)
SCH_A = 8.0 * 128.0 / 0.6931471805599453
SCH_B = 127.0 * 128.0 - 5.0

_CACHE = {}


def _prep_x(xi):
    """Host staging: [56,56,64] -> (xpT, x3) [128, HP, WP] float32."""
    base = np.zeros((C, HP, WP), np.float32)
    base[:, 1:1 + H, 1:1 + W] = np.ascontiguousarray(xi.transpose(2, 0, 1))
    xp = np.zeros((128, HP, WP), np.float32)
    xp[0:C] = base
    xp[C:128, 0:HP - 1] = base[:, 1:HP]
    x3 = np.zeros((128, HP, WP), np.float32)
    x3[0:C] = base
    x3[C:128, :, 0:WP - 1] = base[:, :, 1:WP]
    return xp, x3


def _prep_weights(wq, wk, wv, bq, bv, Wp, bp):
    """Host staging of the conv weights (diag blocks built on-device).

    wtap [128, 2, 5]: per-partition tap values, [:,0,s]=wq, [:,1,s]=wk
      (partition p<64: lower tap of slot s for channel p; p>=64: upper tap).
    stv [128, 64, 5]: the dense v-projection blocks wv[tap]*Wp, laid out
      [p, out_col, slot] to sit contiguously behind the on-device diag.
    bqb1 [128, 2]: col 0 rows 0-63 = bq; col 1 rows 64-127 = bv@Wp + bp.
    """
    wq = np.asarray(wq, np.float32).reshape(9, C)
    wk = np.asarray(wk, np.float32).reshape(9, C)
    wv = np.asarray(wv, np.float32).reshape(9, C)
    Wp = np.asarray(Wp, np.float32)
    wtap = np.zeros((128, 2, 6), np.float32)
    stv = np.zeros((128, C, 5), np.float32)
    for s, (lt, ut) in enumerate(CONV_SLOTS):
        wtap[0:C, 0, s] = wq[lt]
        wtap[0:C, 1, s] = wk[lt]
        stv[0:C, :, s] = wv[lt][:, None] * Wp
        if ut is not None:
            wtap[C:128, 0, s] = wq[ut]
            wtap[C:128, 1, s] = wk[ut]
            stv[C:128, :, s] = wv[ut][:, None] * Wp
    # biases ride along in wtap[:, :, 5]: [0:C, 0, 5] = bq,
    # [C:128, 1, 5] = bv@Wp + bp
    wtap[0:C, 0, 5] = np.asarray(bq, np.float32)
    wtap[C:128, 1, 5] = (np.asarray(bv, np.float32) @ Wp
                         + np.asarray(bp, np.float32))
    return wtap, stv


def _build(level=99):
    import concourse.bacc as bacc
    import concourse.tile as tile
    from concourse import mybir
    from concourse.masks import make_identity
    from concourse.tile import add_dep_helper

    F32 = mybir.dt.float32
    F32R = mybir.dt.float32r
    BF16 = mybir.dt.bfloat16
    AF = mybir.ActivationFunctionType

    nc = bacc.Bacc(None, target_bir_lowering=False, debug=False)

    x_d = nc.dram_tensor("x", [128, HP, WP], F32R, kind="ExternalInput")
    x3_d = nc.dram_tensor("x3", [128, HP, WP], F32R, kind="ExternalInput")
    wtap_d = nc.dram_tensor("wtap", [128, 2, 6], F32, kind="ExternalInput")
    stv_d = nc.dram_tensor("stv", [128, C, 5], F32R, kind="ExternalInput")
    # per-tile padded output: token qt*448 + s*128 + p -> out_d[qt, s*128+p]
    # (s=3 rows 64-127 are never written; host slices them off)
    out_d = nc.dram_tensor("out", [NQT, 4 * 128, E], F32, kind="ExternalOutput")

    # row-chunked image loads, sized for the serial DMA model: rows 0-9 for
    # tile 0's convs up front, then uniform 8-row chunks just ahead of the
    # kv-conv demand curve. x3 rows 0-1 are never read (conv slot 3 reads
    # rows 8ct+2..8ct+9), so its first chunk starts at row 2.
    RCH = [10, 18, 26, 34, 42, 50, HP]

    with tile.TileContext(nc) as tc:
        with tc.tile_pool(name="const", bufs=1) as const, \
             tc.tile_pool(name="big", bufs=1) as big:
            # warmup tile first: DVE memset -> PE p-state ramp starts ~1us in
            # (memset of f32r is not a valid ISA op; memset as f32, matmul
            # reads it through an f32r bitcast)
            wtile = const.tile([128, 128], F32)
            nc.vector.memset(wtile[:], 0.0)

            xpT = big.tile([128, HP, WP], F32R)
            x3 = big.tile([128, HP, WP], F32R)
            wtap = const.tile([128, 2, 6], F32)
            # [p, out_col, slot]: cols 0-63 diag (on-device), v block via DMA
            st_q = const.tile([128, C, 5], F32R)
            st_kv = const.tile([128, 128, 5], F32R)

            # DMA plan: single (sync) queue; the shared DMA engine device is
            # serial in the cost model and HWDGE descriptor generation costs
            # 625ns per dma_start, so both order and COUNT of front DMAs
            # matter. The diag lhsT blocks are built on-device (wtap is 48B
            # vs 3.8KB staged) and the biases ride in wtap, keeping the
            # serial front at 4 descriptors before the row chunks.
            nc.sync.dma_start(wtap[:], wtap_d[:])
            nc.sync.dma_start(xpT[:, 0:RCH[0], :], x_d[:, 0:RCH[0], :])
            nc.sync.dma_start(st_kv[:, C:128, :], stv_d[:])
            nc.sync.dma_start(x3[:, 2:RCH[0], :], x3_d[:, 2:RCH[0], :])
            for ci in range(len(RCH) - 1):
                r0, r1 = RCH[ci], RCH[ci + 1]
                nc.sync.dma_start(xpT[:, r0:r1, :], x_d[:, r0:r1, :])
                nc.sync.dma_start(x3[:, r0:r1, :], x3_d[:, r0:r1, :])

            ident_f = const.tile([128, 128], F32)
            make_identity(nc, ident_f[:])
            ident_b = const.tile([128, 128], BF16)
            nc.vector.tensor_copy(ident_b[:], ident_f[:])

            # two-half diagonal mask M[p, y] = (y == p mod 64) for the
            # on-device diag builds
            dmask = const.tile([128, C], F32)
            nc.gpsimd.memset(dmask[:], 0.0)
            for half in (0, 1):
                nc.gpsimd.affine_select(
                    out=dmask[half * C:(half + 1) * C, :],
                    in_=dmask[half * C:(half + 1) * C, :],
                    compare_op=mybir.AluOpType.not_equal,
                    fill=1.0,
                    base=0,
                    pattern=[[-1, C]],
                    channel_multiplier=1,
                )
            # st_q / st_kv diag blocks: M * per-partition tap value
            for s in range(5):
                nc.vector.tensor_scalar_mul(
                    st_q[:, :, s], dmask[:], wtap[:, 0, s:s + 1])
                nc.vector.tensor_scalar_mul(
                    st_kv[:, 0:C, s], dmask[:], wtap[:, 1, s:s + 1])

            zsc = const.tile([128, 128], F32)
            nc.vector.memset(zsc[:], 0.0)
            ones_f = const.tile([128, NKC], F32)
            nc.vector.memset(ones_f[:], 1.0)

            qT = big.tile([C, N], F32R)            # q^T  [c, token]
            kT = big.tile([C, NPAD], F32R)         # k^T  [c, token], zero pad
            vT = big.tile([128, N], BF16)          # v''^T on partitions 64-127
            v_nat = big.tile([128, NKC, C + 1], BF16)  # [tok%128, chunk, e|1]

            nc.vector.tensor_copy(kT[:, N:NPAD], zsc[0:C, 0:NPAD - N])
            nc.vector.tensor_copy(v_nat[:, :, C], ones_f[:])

            with tc.tile_pool(name="ps2", bufs=2, space="PSUM") as ps2, \
                 tc.tile_pool(name="psSa", bufs=1, space="PSUM") as psSa, \
                 tc.tile_pool(name="psSb", bufs=1, space="PSUM") as psSb, \
                 tc.tile_pool(name="psSd", bufs=1, space="PSUM") as psSd, \
                 tc.tile_pool(name="psC", bufs=1, space="PSUM") as psC, \
                 tc.tile_pool(name="sbP", bufs=2 * NGRP) as sbP, \
                 tc.tile_pool(name="sbPd", bufs=2 * len(DVE_CH)) as sbPd, \
                 tc.tile_pool(name="sbO", bufs=4) as sbO, \
                 tc.tile_pool(name="sbI", bufs=4) as sbI:

                # PE warmup: set pe_busy_start early so the clock ramps to
                # full rate during the image DMA wait
                # pe_busy_start is sticky from the first matmul (no reset on
                # idle), so two early matmuls are enough to start the clock
                # ramp; the PE hits full rate ~3us later regardless of idling
                ptw = ps2.tile([128, NQ], F32, name="ptw", tag="cv")
                for _ in range(2):
                    nc.tensor.matmul(ptw[0:128, 0:128],
                                     wtile[:, 0:128].bitcast(F32R),
                                     wtile[:, 0:128].bitcast(F32R),
                                     start=True, stop=True)

                # ---- incremental emitters ----------------------------------
                st = {"kv": 0, "vn": 0, "pctx": None}

                def conv_matmuls(pdst, lhsT, ct, mwid, slots=range(5)):
                    r0 = ct * 8
                    first = None
                    for i in slots:
                        s = SLOT_ORDER[i]
                        if s < 3:
                            rhs = xpT[:, r0:r0 + 8, s:s + W]
                            lh = lhsT[:, 0:mwid, s]
                        elif s == 3:
                            rhs = x3[:, r0 + 2:r0 + 10, 0:W]
                            lh = lhsT[:, 0:mwid, s]
                        else:
                            rhs = xpT[0:C, r0 + 2:r0 + 10, 2:2 + W]
                            lh = lhsT[0:C, 0:mwid, s]
                        mm = nc.tensor.matmul(pdst[:], lh, rhs,
                                              start=(i == 0), stop=(i == 4))
                        if first is None:
                            first = mm
                    return first

                def emit_vt_add(pkv, ct):
                    nc.vector.tensor_scalar_add(
                        vT[C:128, ct * NQ:(ct + 1) * NQ], pkv[C:128, :],
                        wtap[C:128, 1, 5:6])

                def emit_kv(upto):
                    while st["kv"] <= min(upto, NQT - 1):
                        ct = st["kv"]
                        pkv = ps2.tile([128, NQ], F32, name="pkv", tag="cv")
                        first = conv_matmuls(pkv, st_kv, ct, 128)
                        if False and st.get("qk_dep") is not None:
                            # keep the tile scheduler from hoisting kv-conv 1
                            # ahead of QK group 0 on the PE (its CoreSim DMA
                            # model is optimistic about the x chunk arrival)
                            add_dep_helper(first.ins, st["qk_dep"].ins,
                                           reason="kv conv after QK group")
                        nc.vector.tensor_copy(kT[:, ct * NQ:(ct + 1) * NQ],
                                              pkv[0:C, :])
                        emit_vt_add(pkv, ct)
                        st["kv"] += 1

                def emit_vnat(limit=NKC):
                    # v_nat transposes, batched 4 chunks per PSUM tile /
                    # copy to amortize the DVE PSUM-access overhead
                    top = st["kv"] * NQ
                    while st["vn"] < limit:
                        kc0 = st["vn"]
                        nb = min(4, NKC - kc0)
                        end = kc0 + nb - 1
                        cw_last = min(KC, N - end * KC)
                        if end * KC + cw_last > top:
                            break
                        tp = psC.tile([128, 4, C], BF16, name="tpv",
                                      tag="ctx")
                        for j in range(nb):
                            kc = kc0 + j
                            cw = min(KC, N - kc * KC)
                            nc.tensor.transpose(
                                tp[0:cw, j, :],
                                vT[C:128, kc * KC:kc * KC + cw],
                                ident_b[C:128, C:128])
                        cw = min(KC, N - (kc0 + nb - 1) * KC)
                        if nb == 4 and cw == KC:
                            nc.vector.tensor_copy(
                                v_nat[:, kc0:kc0 + nb, 0:C], tp[:, 0:nb, :])
                        else:
                            for j in range(nb):
                                kc = kc0 + j
                                cw = min(KC, N - kc * KC)
                                nc.vector.tensor_copy(
                                    v_nat[0:cw, kc, 0:C], tp[0:cw, j, :])
                        st["vn"] += nb

                def emit_qconv_slots(pq, qt, slots):
                    r0 = qt * 8
                    for i in slots:
                        s = SLOT_ORDER[i]
                        if s < 3:
                            rhs = xpT[:, r0:r0 + 8, s:s + W]
                            lh = st_q[:, :, s]
                        elif s == 3:
                            rhs = x3[:, r0 + 2:r0 + 10, 0:W]
                            lh = st_q[:, :, s]
                        else:
                            rhs = xpT[0:C, r0 + 2:r0 + 10, 2:2 + W]
                            lh = st_q[0:C, :, s]
                        nc.tensor.matmul(pq[:], lh, rhs,
                                         start=(i == 0), stop=(i == 4))

                def emit_qcopy(pq, qt):
                    if qt == 0:
                        # ACT is idle before the first exp; DVE is busy with
                        # lhsT builds
                        nc.scalar.add(qT[:, qt * NQ:(qt + 1) * NQ], pq[:],
                                      wtap[0:C, 0, 5:6])
                    else:
                        nc.vector.tensor_scalar_add(
                            qT[:, qt * NQ:(qt + 1) * NQ], pq[:], wtap[0:C, 0, 5:6])

                def emit_av_chunks(tiles, s, pctx, chunks):
                    # chunk-matmuls of one q-subtile into accumulator pctx
                    pT_tiles, dve_tiles = tiles
                    s0 = s * 128
                    sw = min(128, NQ - s0)
                    for kc in chunks:
                        cw = 64 if kc == NKC - 1 else 128
                        loc = CH_LOC[kc]
                        if loc[0] == "dve":
                            lhsT = dve_tiles[loc[1]][0:cw, s0:s0 + sw].bitcast(
                                mybir.dt.bfloat16)
                        else:
                            lhsT = pT_tiles[loc[1]][0:cw, loc[2], s0:s0 + sw]
                        nc.tensor.matmul(
                            pctx[0:sw, :],
                            lhsT,
                            v_nat[0:cw, kc, :],
                            start=(kc == 0), stop=(kc == NKC - 1))

                def emit_av_batch(pT_tiles, s, half):
                    # 25 chunk-matmuls of one q-subtile, split in two halves;
                    # one pending psum group at a time (zero-region rule)
                    if half == 0:
                        st["pctx"] = psC.tile([128, C + 1], F32,
                                              name="pctx", tag="ctx")
                    chunks = range(0, 13) if half == 0 else range(13, NKC)
                    emit_av_chunks(pT_tiles, s, st["pctx"], chunks)

                def emit_norm_sub(pctx, qt, s, on_act=False, swdge=False):
                    # normalize into the tile's gathered [128, 4, E] buffer;
                    # subs 1 and 3 each trigger a half-tile store DMA. The
                    # last tile runs half the muls on ACT (idle by then) and
                    # its second store through SWDGE so the two descriptor
                    # generations overlap.
                    s0 = s * 128
                    sw = min(128, NQ - s0)
                    inv = sbI.tile([128, 1], F32, name="inv", tag="inv")
                    nc.vector.reciprocal(inv[0:sw, :], pctx[0:sw, C:C + 1])
                    if s == 0:
                        st["osb4"] = sbO.tile([128, 4, E], F32, name="osb4",
                                              tag="out")
                    osb4 = st["osb4"]
                    if on_act:
                        nc.scalar.activation(
                            osb4[0:sw, s, :], pctx[0:sw, 0:C],
                            AF.Copy, scale=inv[0:sw, 0:1])
                    else:
                        nc.vector.tensor_scalar_mul(
                            osb4[0:sw, s, :], pctx[0:sw, 0:C], inv[0:sw, 0:1])
                    if s in (1, 3):
                        r0 = (s - 1) * 128
                        dma = nc.gpsimd.dma_start if swdge else nc.sync.dma_start
                        dma(out_d[qt, r0:r0 + 256, :].rearrange(
                                "(s p) e -> p s e", p=128),
                            osb4[:, s - 1:s + 1, :])

                def flush_prev(prev, g):
                    # AV batches one group later than minimal so the psC WAR
                    # (sub start vs previous norm) never stalls the PE
                    if prev is None or g < 1 or g > 8:
                        return
                    qt_prev, pT_tiles = prev
                    emit_av_batch(pT_tiles, (g - 1) // 2, (g - 1) % 2)
                    if (g - 1) % 2 == 1:
                        emit_norm_sub(st["pctx"], qt_prev, (g - 1) // 2)

                # ---- lead-in: q-conv(0) + kv(0) ----------------------------
                # PE order tuned for the kT critical path: xpT-only slots of
                # both convs first, then kv's x3 slot (so the kT copy can
                # start ASAP), then q's x3 slot + bias copy. vT add deferred
                # past QK g0 (engine-counter waits would otherwise put it on
                # QK's critical path).
                tap_sched = {0: (0,), 1: (1,), 2: (2,), 3: (3,), 4: (4,)}
                if level >= 2:
                    pq = ps2.tile([C, NQ], F32, name="pq", tag="cv")
                    emit_qconv_slots(pq, 0, range(4))
                    pkv0 = ps2.tile([128, NQ], F32, name="pkv", tag="cv")
                    conv_matmuls(pkv0, st_kv, 0, 128, slots=range(4))
                    conv_matmuls(pkv0, st_kv, 0, 128, slots=(4,))
                    nc.vector.tensor_copy(kT[:, 0:NQ], pkv0[0:C, :])
                    emit_qconv_slots(pq, 0, (4,))
                    emit_qcopy(pq, 0)
                    st["kv"] = 1
                    st["pkv0"] = pkv0

                prev = None
                last = NQT - 1
                for qt in range(NQT if level >= 5 else 0):
                    q0 = qt * NQ
                    pq_next = None
                    pT_tiles = []
                    dve_tiles = []
                    tiles = (pT_tiles, dve_tiles)
                    pc6 = [None, None, None, None]  # last-tile accumulators
                    if qt == last and level >= 6:
                        pc6[0] = ps2.tile([128, C + 1], F32, name="pc6a",
                                          tag="cv")
                        pc6[1] = ps2.tile([128, C + 1], F32, name="pc6b",
                                          tag="cv")
                    for g in range(NGRP):
                        pool = psSa if g % 2 == 0 else psSb
                        ps_s = pool.tile([128, 2, 512], F32, name="ps_s",
                                         tag="sa" if g % 2 == 0 else "sb")
                        import contextlib
                        prio = (tc.high_priority() if qt == 0 and g < 2
                                else contextlib.nullcontext())
                        with prio:
                            for j, kc in enumerate(ACT_GRP[g]):
                                qk_mm = nc.tensor.matmul(
                                    ps_s[:, j, 0:NQ],
                                    kT[:, kc * KC:(kc + 1) * KC],
                                    qT[:, q0:q0 + NQ],
                                    start=True, stop=True)
                        if qt == 0:
                            # tile 0: kv convs + their kT/vT copies go FIRST
                            # so the dve sch (waiting on its QK) never
                            # head-of-line-blocks them on the DVE queue
                            if g == 0:
                                emit_vt_add(st["pkv0"], 0)
                            if g + 1 < NGRP:
                                emit_kv(KV_NEED[g + 1])
                        if g in DVE_AFTER:
                            # DVE-destined chunk: own 1-bank score lane +
                            # Schraudolph exp, fully decoupled from the ACT
                            # group tiles so the scheduler never chains them
                            kc = DVE_CH[DVE_AFTER[g]]
                            psd = psSd.tile([128, 512], F32, name="psd",
                                            tag="sd")
                            qk_mm = nc.tensor.matmul(
                                psd[:, 0:NQ],
                                kT[:, kc * KC:(kc + 1) * KC],
                                qT[:, q0:q0 + NQ],
                                start=True, stop=True)
                            pTd = sbPd.tile([128, NQ], mybir.dt.int16,
                                            name="pTd", tag="pd")
                            nc.vector.tensor_scalar(
                                pTd[:, :], psd[:, 0:NQ], SCH_A, SCH_B,
                                mybir.AluOpType.mult, mybir.AluOpType.add)
                            dve_tiles.append(pTd)
                        st["qk_dep"] = qk_mm
                        flush_prev(prev, g)
                        if qt == last and level >= 6:
                            if g == 8:
                                # tile-5 AV fully flushed; psC free for sub 3
                                pc6[3] = psC.tile([128, C + 1], F32,
                                                  name="pc6d", tag="ctx")
                            if g >= 1:
                                emit_av_chunks(tiles, 0, pc6[0], STEP[g - 1])
                                emit_av_chunks(tiles, 1, pc6[1], STEP[g - 1])
                            if g == NGRP - 1:
                                # catch-up: subs 2/3 over steps 0..8
                                hi = min(STEP[g])
                                emit_av_chunks(tiles, 2, pc6[2], range(0, hi))
                                emit_av_chunks(tiles, 3, pc6[3], range(0, hi))
                        if qt == 0:
                            # v_nat transposes wait for tile 0's PE
                            # congestion to pass (not needed until tile 1's
                            # AV batches)
                            if g == NGRP - 1 and qt + 1 < NQT:
                                emit_vnat(16)
                                pq_next = ps2.tile([C, NQ], F32,
                                                   name="pq", tag="cv")
                                emit_qconv_slots(pq_next, 1, range(5))
                                emit_qcopy(pq_next, 1)
                        elif qt + 1 < NQT:
                            if qt == 1 and g == 0:
                                emit_vnat()
                            if g in tap_sched:
                                if pq_next is None:
                                    pq_next = ps2.tile([C, NQ], F32,
                                                       name="pq", tag="cv")
                                emit_qconv_slots(pq_next, qt + 1, tap_sched[g])
                            if g == 5:
                                emit_qcopy(pq_next, qt + 1)
                        ng = len(ACT_GRP[g])
                        pTt = sbP.tile([128, 2, NQ], BF16, name="pTt", tag="p")
                        nc.scalar.activation(
                            pTt[:, 0:ng, :], ps_s[:, 0:ng, 0:NQ],
                            AF.Exp, scale=8.0)
                        pT_tiles.append(pTt)
                        if qt == last and level >= 6 and g == 8:
                            # psSa's last score group read; slot free for sub 2
                            pc6[2] = psSa.tile([128, C + 1], F32,
                                               name="pc6c", tag="sa")
                    if level >= 6:
                        if qt < last:
                            prev = (qt, tiles)
                        else:
                            # tail: per sub, last step's chunks then norm,
                            # so the half-tile stores fire ASAP
                            for s in range(4):
                                emit_av_chunks(tiles, s, pc6[s],
                                               STEP[NGRP - 1])
                                # the earlier-ready half rides the slower
                                # SWDGE gen; the later half takes HWDGE
                                emit_norm_sub(pc6[s], qt, s,
                                              on_act=(s in (1, 3)),
                                              swdge=(s == 1))

    nc.compile()
    return nc


def _get_nc():
    if "nc" not in _CACHE:
        _CACHE["nc"] = _build()
    return _CACHE["nc"]


def kernel(x, wq, bq, wk, bk, wv, bv, Wp, bp):
    from concourse.bass_utils import run_bass_kernel_spmd

    nc = _get_nc()
    x = np.asarray(x, dtype=np.float32)
    wtap, stv = _prep_weights(wq, wk, wv, bq, bv, Wp, bp)
    shared = {"wtap": wtap, "stv": stv}
    in_maps = []
    for i in range(NCORES):
        xp, x3 = _prep_x(x[i])
        in_maps.append(dict(shared, x=xp, x3=x3))
    res = run_bass_kernel_spmd(nc, in_maps, core_ids=list(range(NCORES)))
    out = np.stack([
        np.asarray(res.results[i]["out"])
        .reshape(NQT, 4 * 128, E)[:, 0:NQ, :].reshape(H, W, E)
        for i in range(NCORES)
    ])
    return out
